# revision 55
# baseline (speedup 1.0000x reference)
"""HSIViT forward on 8 Trainium2 NeuronCores.

Sharding: pure data parallel — batch B=8, one batch item per core, no
collectives. Each core runs the full 12-layer ViT on its (512, 768)
token activations and emits its (100,) logits row.

Host-side prep (numpy, not counted in HW exec time):
  - patch cubes extracted + transposed per batch item (xpT [512, 512])
  - all weights transposed to [c_in, c_out] for the PE's lhsT layout
  - LN1/LN2 scale+bias folded into q/k/v and fc1 weights+biases
  - final feature-LN scale/bias folded into the classifier head
  - q/k/v/proj/MLP weights cast to bf16; patch embed + head ride fp32r

Scheduling notes (all aimed at keeping the PE queue dense so the
Tensor engine stays in its high p-state):
  - LN rstd via one ACT Rsqrt op; standardized rows evicted bf16 so the
    PE transposes run at 1 cyc/row; per-tile LN chains overlap the
    previous phase's matmuls (fc2/proj emit token-tile-ordered).
  - attention: S^T per head in 4 psum chunks -> ACT exp (bf16) -> AV
    with a ones-column in V producing the softmax denominator; AV lags
    scores by 2 heads and the V projection is interleaved after S0/S1
    so the PE has independent work while ACT exps drain.
  - softmax reciprocal via the custom-DVE fast-approx op; denominator
    broadcast on GpSimd; normalization fused into the oT eviction.
  - proj/fc2 biases pre-added into h during PE-busy windows so the
    post-psum residual is a single vector add.
"""

import os
import sys

import numpy as np

for _p in ("/opt/trn_rl_repo", "/root/.axon_site/_ro/trn_rl_repo"):
    if _p not in sys.path and os.path.isdir(_p):
        sys.path.insert(0, _p)

import ml_dtypes  # noqa: E402

import concourse.bass as bass  # noqa: E402,F401
import concourse.mybir as mybir  # noqa: E402
import concourse.tile as tile  # noqa: E402
from concourse import bacc  # noqa: E402
from concourse.bass_utils import run_bass_kernel_spmd  # noqa: E402
from concourse.masks import make_identity  # noqa: E402

F32 = mybir.dt.float32
F32R = mybir.dt.float32r
BF16 = mybir.dt.bfloat16
AF = mybir.ActivationFunctionType
OP = mybir.AluOpType
AX = mybir.AxisListType

DEPTH, C, NH, HD = 12, 768, 12, 64
NTOK, PVEC = 512, 512  # tokens, patch vector (8*8*8)
FF = 3072
NCLS = 100
TB, SP = 8, 64  # band groups, spatial positions
FD = TB * C  # 6144 final feature dim
SCALE = HD**-0.5
EPS = 1e-5

CB_PER_LAYER = 36  # qb 6 + kb 6 + f1b 24 columns
CB_W1 = DEPTH * CB_PER_LAYER  # head-weight column sums
CB_HB = CB_W1 + 1  # folded head bias
CB_COLS = CB_HB + 1

bf16 = ml_dtypes.bfloat16


def _build():
    nc = bacc.Bacc(None, target_bir_lowering=False, debug=False)

    d_xpt = nc.dram_tensor("xpt", [PVEC, NTOK], BF16, kind="ExternalInput")
    d_pos2 = nc.dram_tensor("pos2", [NTOK, C], F32, kind="ExternalInput")
    d_pwt = nc.dram_tensor("pwt", [PVEC, C], BF16, kind="ExternalInput")
    d_wq = nc.dram_tensor("wq", [DEPTH, C, C], BF16, kind="ExternalInput")
    d_wk = nc.dram_tensor("wk", [DEPTH, C, C], BF16, kind="ExternalInput")
    d_wv = nc.dram_tensor("wv", [DEPTH, C, C], BF16, kind="ExternalInput")
    d_wp = nc.dram_tensor("wp", [DEPTH, C, C], BF16, kind="ExternalInput")
    d_w1 = nc.dram_tensor("w1", [DEPTH, C, FF], BF16, kind="ExternalInput")
    d_w2 = nc.dram_tensor("w2", [DEPTH, FF, C], BF16, kind="ExternalInput")
    d_cb = nc.dram_tensor("cb", [128, CB_COLS], F32, kind="ExternalInput")
    d_rb = nc.dram_tensor("rb", [DEPTH, 3, C], F32, kind="ExternalInput")
    d_hwt = nc.dram_tensor("hwt", [FD, NCLS], BF16, kind="ExternalInput")
    d_out = nc.dram_tensor("out", [NCLS], F32, kind="ExternalOutput")

    from contextlib import ExitStack

    with tile.TileContext(nc) as tc:
        with ExitStack() as ctx:
            ep = ctx.enter_context
            const = ep(tc.tile_pool(name="const", bufs=1))
            hpool = ep(tc.tile_pool(name="hpool", bufs=4))
            arow_p = ep(tc.tile_pool(name="arow", bufs=4))
            aT_p = ep(tc.tile_pool(name="atp", bufs=6))
            a2T_p = ep(tc.tile_pool(name="a2tp", bufs=6))
            qT_p = ep(tc.tile_pool(name="qtp", bufs=6))
            kT_p = ep(tc.tile_pool(name="ktp", bufs=6))
            vx_p = ep(tc.tile_pool(name="vxp", bufs=4))
            ex_p = ep(tc.tile_pool(name="exp", bufs=14))
            oT_p = ep(tc.tile_pool(name="otp", bufs=6))
            gT_p = ep(tc.tile_pool(name="gtp", bufs=24))
            wqkv_p = ep(tc.tile_pool(name="wqkv", bufs=12))
            patch_p = ep(tc.tile_pool(name="patchp", bufs=4))
            hw_p = ep(tc.tile_pool(name="hwp", bufs=4))
            w1_p = ep(tc.tile_pool(name="w1p", bufs=9))
            w2_p = ep(tc.tile_pool(name="w2p", bufs=24))
            bc_p = ep(tc.tile_pool(name="bcp", bufs=2))
            rcp_p = ep(tc.tile_pool(name="rcpp", bufs=2))
            sm_p = ep(tc.tile_pool(name="smp", bufs=8))
            sm512_p = ep(tc.tile_pool(name="sm512", bufs=2))
            ftmp_p = ep(tc.tile_pool(name="ftmp", bufs=6))
            sq_p = ep(tc.tile_pool(name="sqp", bufs=2))
            mm_ps = ep(tc.tile_pool(name="mmps", bufs=3, space="PSUM"))
            st_ps = ep(tc.tile_pool(name="stps", bufs=3, space="PSUM"))
            tp_ps = ep(tc.tile_pool(name="tpps", bufs=2, space="PSUM"))

            ident = const.tile([128, 128], F32, tag="ident", name="ident")
            make_identity(nc, ident)
            identB = const.tile([128, 128], BF16, tag="identB", name="identB")
            nc.scalar.copy(identB[:], ident[:])
            ones0 = const.tile([128, 1], F32, tag="ones0", name="ones0")
            nc.vector.memset(ones0[:], 1.0)
            onesB = const.tile([128, 1], BF16, tag="onesB", name="onesB")
            nc.scalar.copy(onesB[:], ones0[:])
            eps = const.tile([128, 1], F32, tag="eps", name="eps")
            nc.vector.memset(eps[:], EPS)
            cb = const.tile([128, CB_COLS], F32, tag="cb", name="cb")
            nc.sync.dma_start(out=cb[:], in_=d_cb[:])

            h = []
            for t in range(4):
                ht = hpool.tile([128, C], F32, tag="h", name=f"h{t}")
                h.append(ht)

            def emit_stats0(t, tag):
                """First bn_stats half — emitted as soon as h[t][:, 0:384]
                is final (after the n=0 residual add) so it overlaps the
                n=1 psum matmuls."""
                st6 = sm_p.tile([128, 12], F32, tag="st6", name=f"st6_{tag}{t}")
                nc.vector.bn_stats(st6[:, 0:6], h[t][:, 0:384])
                return st6

            def emit_ln_rest(t, tag, st6):
                """Second stats half + (x - mean) * rsqrt(var + eps), bf16."""
                nc.vector.bn_stats(st6[:, 6:12], h[t][:, 384:768])
                mv = sm_p.tile([128, 2], F32, tag="mv", name=f"mv{tag}{t}")
                nc.vector.bn_aggr(mv[:], st6.rearrange("p (g s) -> p g s", g=2))
                std = sm_p.tile([128, 1], F32, tag="std", name=f"std{tag}{t}")
                nc.scalar.activation(std[:], mv[:, 1:2], AF.Sqrt, bias=eps[:])
                rstd = sm_p.tile([128, 1], F32, tag="rstd", name=f"rstd{tag}{t}")
                nc.vector.reciprocal_approx_fast(out=rstd[:], in_=std[:])
                at = arow_p.tile([128, C], BF16, tag="ar", name=f"ar{tag}{t}")
                nc.vector.tensor_scalar(
                    at[:], h[t], mv[:, 0:1], rstd[:], op0=OP.subtract, op1=OP.mult
                )
                return at

            # ---- patch embed: h = pos(+patch_b) + xp @ patch_w.T ----
            # pos is DMA'd straight into h, psum added on top; t-major with
            # layer 0's LN1 emitted inline so its chains overlap the PE.
            xpt = []
            pwt = []
            for kc in range(4):
                xt = aT_p.tile([128, NTOK], BF16, tag="at", name=f"xpt{kc}")
                nc.sync.dma_start(out=xt[:], in_=d_xpt[kc * 128 : (kc + 1) * 128, :])
                xpt.append(xt)
                wt = patch_p.tile([128, C], BF16, tag="pw", name=f"pwt{kc}")
                nc.sync.dma_start(out=wt[:], in_=d_pwt[kc * 128 : (kc + 1) * 128, :])
                pwt.append(wt)
            a_rows = []
            for t in range(4):
                nc.sync.dma_start(out=h[t][:], in_=d_pos2[t * 128 : (t + 1) * 128, :])
                st6 = None
                for n in range(2):
                    ns = slice(n * 384, (n + 1) * 384)
                    ps = mm_ps.tile([128, 512], F32, tag="mm", name=f"pep{t}{n}")
                    for kc in range(4):
                        nc.tensor.matmul(
                            ps[:, :384],
                            xpt[kc][:, t * 128 : (t + 1) * 128],
                            pwt[kc][:, ns],
                            start=(kc == 0),
                            stop=(kc == 3),
                        )
                    nc.vector.tensor_tensor(h[t][:, ns], h[t][:, ns], ps[:, :384], op=OP.add)
                    if n == 0:
                        st6 = emit_stats0(t, "a0_")
                a_rows.append(emit_ln_rest(t, "a0_", st6))

            def transpose_pass(rows, t_list, col0, outs, tag2):
                """Transpose the given token tiles' blocks into cols
                [col0 : col0 + 128*len(t_list)] of the 6 col tiles.
                Evictions alternate ACT/DVE so the tail drains on two
                queues."""
                w = 128 * len(t_list)
                for cc in range(6):
                    ps = tp_ps.tile([128, 512], BF16, tag="tp", name=f"tp{tag2}{cc}")
                    for ti, t in enumerate(t_list):
                        nc.tensor.transpose(
                            ps[:, ti * 128 : (ti + 1) * 128],
                            rows[t][:, cc * 128 : (cc + 1) * 128],
                            identB[:],
                        )
                    dst = outs[cc][:, col0 : col0 + w]
                    if cc % 2 == 0:
                        nc.scalar.copy(dst, ps[:, 0:w])
                    else:
                        nc.vector.tensor_copy(dst, ps[:, 0:w])

            def transpose_cols(rows, dst_pool, dst_tag):
                outs = [
                    dst_pool.tile([128, NTOK], BF16, tag=dst_tag, name=f"{dst_tag}{cc}")
                    for cc in range(6)
                ]
                transpose_pass(rows, (0, 1, 2, 3), 0, outs, dst_tag)
                return outs

            def bcast_row(i, j, tag):
                """rb[i, j] (768,) -> [128, 768] partition-broadcast tile."""
                src = sm512_p.tile([1, C], F32, tag="rbs", name=f"rbs{i}_{j}")
                nc.sync.dma_start(out=src[:], in_=d_rb[i, j])
                bt = bc_p.tile([128, C], F32, tag="bc", name=f"{tag}{i}")
                nc.gpsimd.partition_broadcast(bt[:], src[:])
                return bt

            for i in range(DEPTH):
                cb0 = i * CB_PER_LAYER
                # ---- transpose LN1 rows (tokens 0..383 first) ----
                aT = [
                    aT_p.tile([128, NTOK], BF16, tag="at", name=f"at{cc}")
                    for cc in range(6)
                ]
                transpose_pass(a_rows, (0, 1, 2), 0, aT, "at1_")

                # ---- q/k projections -> col layout [c_out, t], bf16;
                # two token passes: cols 0:384 run while tile 3's LN
                # finishes, then the t3 transpose pass, then cols 384:512 ----
                qk_w = []
                for (dw, tg) in ((d_wq, "qw"), (d_wk, "kw")):
                    wts = []
                    for kc in range(6):
                        wt = wqkv_p.tile([128, C], BF16, tag="wqkv", name=f"{tg}{kc}")
                        nc.sync.dma_start(out=wt[:], in_=dw[i, kc * 128 : (kc + 1) * 128, :])
                        wts.append(wt)
                    qk_w.append(wts)
                qT = [qT_p.tile([128, NTOK], BF16, tag="qt", name=f"qt{mc}") for mc in range(6)]
                kT = [kT_p.tile([128, NTOK], BF16, tag="kt", name=f"kt{mc}") for mc in range(6)]
                for cs, ce in ((0, 384), (384, 512)):
                    if cs == 384:
                        transpose_pass(a_rows, (3,), 384, aT, "at2_")
                    w = ce - cs
                    for mc in range(6):
                        for (wts, outs, base, tg) in (
                            (qk_w[0], qT, cb0, "qt"),
                            (qk_w[1], kT, cb0 + 6, "kt"),
                        ):
                            ps = mm_ps.tile([128, 512], F32, tag="mm", name=f"{tg}p{mc}")
                            for kc in range(6):
                                nc.tensor.matmul(
                                    ps[:, 0:w],
                                    wts[kc][:, mc * 128 : (mc + 1) * 128],
                                    aT[kc][:, cs:ce],
                                    start=(kc == 0),
                                    stop=(kc == 5),
                                )
                            nc.vector.tensor_scalar_add(
                                outs[mc][:, cs:ce], ps[:, 0:w],
                                cb[:, base + mc : base + mc + 1],
                            )

                # ---- attention, software-pipelined with the v projection ----
                vbB = bcast_row(i, 0, "vb")
                pbB = bcast_row(i, 1, "pb")
                vwts = []
                for kc in range(6):
                    wt = wqkv_p.tile([128, C], BF16, tag="wqkv", name=f"vw{kc}")
                    nc.sync.dma_start(out=wt[:], in_=d_wv[i, kc * 128 : (kc + 1) * 128, :])
                    vwts.append(wt)

                ex_all = [None] * NH
                po_all = [None] * NH
                oT = []
                for cc in range(6):
                    ot = oT_p.tile([128, NTOK], BF16, tag="ot", name=f"ot{cc}")
                    oT.append(ot)

                def emit_scores(hh):
                    pb_ = (hh % 2) * 64
                    qh = qT[hh // 2][pb_ : pb_ + 64, :]
                    kh = kT[hh // 2][pb_ : pb_ + 64, :]
                    exs = []
                    for j in range(4):
                        ps = st_ps.tile([128, 512], F32, tag="st", name=f"st{hh}_{j}")
                        nc.tensor.matmul(
                            ps[:],
                            kh[:, j * 128 : (j + 1) * 128],
                            qh,
                            start=True,
                            stop=True,
                        )
                        ex = ex_p.tile([128, NTOK], BF16, tag="ex", name=f"ex{hh}_{j}")
                        nc.scalar.activation(ex[:], ps[:], AF.Exp, scale=SCALE)
                        exs.append(ex)
                    ex_all[hh] = exs

                def emit_av(hh):
                    po = mm_ps.tile([128, 512], F32, tag="mm", name=f"po{hh}")
                    for j in range(4):
                        nc.tensor.matmul(
                            po[0:65, :],
                            v_ext[j].rearrange("p (h d) -> p h d", h=NH)[:, hh, :],
                            ex_all[hh][j][:],
                            start=(j == 0),
                            stop=(j == 3),
                        )
                    po_all[hh] = po
                    pb_ = (hh % 2) * 64
                    # custom-DVE ops misread PSUM; bounce the denom row to SBUF
                    den = sm512_p.tile([1, NTOK], F32, tag="rcp", name=f"den{hh}")
                    nc.vector.tensor_copy(den[:], po[64:65, :])
                    rcp = sm512_p.tile([1, NTOK], F32, tag="rcp", name=f"rcp{hh}")
                    nc.vector.reciprocal_approx_fast(out=rcp[:], in_=den[:])
                    rcpB = rcp_p.tile([64, NTOK], F32, tag="rb", name=f"rcpB{hh}")
                    nc.gpsimd.partition_broadcast(rcpB[:], rcp[:])
                    nc.vector.tensor_tensor(
                        oT[hh // 2][pb_ : pb_ + 64, :], po[0:64, :], rcpB[:], op=OP.mult
                    )

                def emit_vgroup(t):
                    vx = vx_p.tile([128, NH * (HD + 1)], BF16, tag="vx", name=f"vx{t}")
                    vxh = vx.rearrange("p (h d) -> p h d", h=NH)
                    for n in range(2):
                        ps = mm_ps.tile([128, 512], F32, tag="mm", name=f"vp{t}{n}")
                        for kc in range(6):
                            nc.tensor.matmul(
                                ps[:, :384],
                                aT[kc][:, t * 128 : (t + 1) * 128],
                                vwts[kc][:, n * 384 : (n + 1) * 384],
                                start=(kc == 0),
                                stop=(kc == 5),
                            )
                        nc.vector.tensor_tensor(
                            vxh[:, n * 6 : (n + 1) * 6, 0:HD],
                            ps[:, :384].rearrange("p (g d) -> p g d", g=6),
                            vbB[:, n * 384 : (n + 1) * 384].rearrange("p (g d) -> p g d", g=6),
                            op=OP.add,
                        )
                    nc.vector.memset(vxh[:, :, HD : HD + 1], 1.0)
                    v_ext.append(vx)

                # pipeline: S0 S1 [v x4] S2 AV0 S3 AV1 ... S11 AV9 AV10 AV11
                # — the v projection fills the PE while the first exps drain
                # on ACT; AV lags scores by 2 heads to hide exp latency.
                v_ext = []
                emit_scores(0)
                emit_scores(1)
                for t in range(4):
                    emit_vgroup(t)
                for t in range(4):
                    nc.vector.tensor_tensor(h[t][:], h[t][:], pbB[:], op=OP.add)
                for hh in range(2, NH):
                    emit_scores(hh)
                    emit_av(hh - 2)
                emit_av(NH - 2)
                emit_av(NH - 1)

                # ---- output projection + residual (pb pre-added), t-major
                # with LN2 emitted inline per tile ----
                pwts = []
                for kc in range(6):
                    wt = wqkv_p.tile([128, C], BF16, tag="wqkv", name=f"pw{kc}")
                    nc.sync.dma_start(out=wt[:], in_=d_wp[i, kc * 128 : (kc + 1) * 128, :])
                    pwts.append(wt)
                a2_rows = []
                for t in range(4):
                    st6 = None
                    for n in range(2):
                        ns = slice(n * 384, (n + 1) * 384)
                        ps = mm_ps.tile([128, 512], F32, tag="mm", name=f"prj{t}{n}")
                        for kc in range(6):
                            nc.tensor.matmul(
                                ps[:, :384],
                                oT[kc][:, t * 128 : (t + 1) * 128],
                                pwts[kc][:, ns],
                                start=(kc == 0),
                                stop=(kc == 5),
                            )
                        nc.vector.tensor_tensor(h[t][:, ns], h[t][:, ns], ps[:, :384], op=OP.add)
                        if n == 0:
                            st6 = emit_stats0(t, f"b{i}_")
                    a2_rows.append(emit_ln_rest(t, f"b{i}_", st6))
                a2T = [
                    a2T_p.tile([128, NTOK], BF16, tag="a2t", name=f"a2t{cc}")
                    for cc in range(6)
                ]
                transpose_pass(a2_rows, (0, 1, 2), 0, a2T, "a2t1_")

                # ---- fc1 + gelu -> gT col layout [j, t] bf16; quarter-sized
                # w1 tiles with a deep ring so the next quarter's DMA hides
                # under this quarter's matmuls; the first 0:384 pass hides
                # tile 3's LN + transpose ----
                f2bB = bcast_row(i, 2, "fb")
                gT = [gT_p.tile([128, NTOK], BF16, tag="gt", name=f"gt{m}") for m in range(24)]
                for quarter in range(4):
                    wts = []
                    for kc in range(6):
                        wt = w1_p.tile([128, FF // 4], BF16, tag="w1", name=f"w1_{quarter}_{kc}")
                        nc.sync.dma_start(
                            out=wt[:],
                            in_=d_w1[
                                i,
                                kc * 128 : (kc + 1) * 128,
                                quarter * (FF // 4) : (quarter + 1) * (FF // 4),
                            ],
                        )
                        wts.append(wt)
                    for cs, ce in ((0, 384), (384, 512)):
                        if cs == 384 and quarter == 0:
                            transpose_pass(a2_rows, (3,), 384, a2T, "a2t2_")
                        w = ce - cs
                        for mh in range(6):
                            m = quarter * 6 + mh
                            ps = mm_ps.tile([128, 512], F32, tag="mm", name=f"f1p{m}")
                            for kc in range(6):
                                nc.tensor.matmul(
                                    ps[:, 0:w],
                                    wts[kc][:, mh * 128 : (mh + 1) * 128],
                                    a2T[kc][:, cs:ce],
                                    start=(kc == 0),
                                    stop=(kc == 5),
                                )
                            nc.scalar.activation(
                                gT[m][:, cs:ce], ps[:, 0:w], AF.Gelu,
                                bias=cb[:, cb0 + 12 + m : cb0 + 13 + m],
                            )
                        if quarter == 0 and cs == 0:
                            # f2b pre-add rides the fc1 window
                            for t in range(4):
                                nc.vector.tensor_tensor(h[t][:], h[t][:], f2bB[:], op=OP.add)

                # ---- fc2 + residual (f2b pre-added); t-major with the NEXT
                # layer's LN1 emitted inline per tile (the LN chains overlap
                # the remaining tiles' fc2 matmuls) ----
                w2ts = []
                for jc in range(24):
                    wt = w2_p.tile([128, C], BF16, tag="w2", name=f"w2_{jc}")
                    nc.sync.dma_start(out=wt[:], in_=d_w2[i, jc * 128 : (jc + 1) * 128, :])
                    w2ts.append(wt)
                a_rows = []
                for t in range(4):
                    st6 = None
                    for n in range(2):
                        ns = slice(n * 384, (n + 1) * 384)
                        ps = mm_ps.tile([128, 512], F32, tag="mm", name=f"f2p{t}{n}")
                        for jc in range(24):
                            nc.tensor.matmul(
                                ps[:, :384],
                                gT[jc][:, t * 128 : (t + 1) * 128],
                                w2ts[jc][:, ns],
                                start=(jc == 0),
                                stop=(jc == 23),
                            )
                        nc.vector.tensor_tensor(h[t][:, ns], h[t][:, ns], ps[:, :384], op=OP.add)
                        if n == 0 and i < DEPTH - 1:
                            st6 = emit_stats0(t, f"a{i + 1}_")
                    if i < DEPTH - 1:
                        a_rows.append(emit_ln_rest(t, f"a{i + 1}_", st6))

            # ---- final: transpose h, feature-LN stats, head ----
            # Per spatial s, feat[s, :] is LN'd over f in [0, 6144) with the
            # LN scale/bias already folded into hwt/head_b. Standardization is
            # folded PAST the head matmul:
            #   logits[n] = (1/64) sum_s rstd[s]*G[n,s]
            #             - (1/64)(sum_s rstd[s]*mean[s]) * W1[n] + head_b'[n]
            # with G = hwt^T @ featT and W1[n] = sum_f hwt[f, n].
            hb = []
            for t in range(4):
                hbt = arow_p.tile([128, C], BF16, tag="ar", name=f"hb{t}")
                nc.scalar.copy(hbt[:], h[t][:])
                hb.append(hbt)
            hT = transpose_cols(hb, ftmp_p, "ht")
            ps_s = st_ps.tile([128, 512], F32, tag="st", name="ps_s")
            ps_q = st_ps.tile([128, 512], F32, tag="st", name="ps_q")
            for cc in range(6):
                s = sq_p.tile([128, NTOK], BF16, tag="sq", name=f"sq{cc}")
                nc.scalar.activation(s[:], hT[cc][:], AF.Square)
                for tb in range(TB):
                    idx = cc * TB + tb
                    nc.tensor.matmul(
                        ps_s[0:1, 0:SP],
                        onesB[:],
                        hT[cc][:, tb * SP : (tb + 1) * SP],
                        start=(idx == 0),
                        stop=(idx == 47),
                    )
                for tb in range(TB):
                    idx = cc * TB + tb
                    nc.tensor.matmul(
                        ps_q[0:1, 0:SP],
                        onesB[:],
                        s[:, tb * SP : (tb + 1) * SP],
                        start=(idx == 0),
                        stop=(idx == 47),
                    )
            mean = sm512_p.tile([1, SP], F32, tag="rbs", name="mean")
            nc.vector.tensor_scalar_mul(mean[:], ps_s[0:1, 0:SP], 1.0 / FD)
            msq = sm512_p.tile([1, SP], F32, tag="rbs", name="msq")
            nc.vector.tensor_scalar_mul(msq[:], ps_q[0:1, 0:SP], 1.0 / FD)
            mm2 = sm512_p.tile([1, SP], F32, tag="rcp", name="mm2")
            nc.vector.tensor_tensor(mm2[:], mean[:], mean[:], op=OP.mult)
            var = sm512_p.tile([1, SP], F32, tag="rcp", name="var")
            nc.vector.tensor_tensor(var[:], msq[:], mm2[:], op=OP.subtract)
            stdf = sm512_p.tile([1, SP], F32, tag="rcp", name="stdf")
            nc.scalar.activation(stdf[:], var[:], AF.Sqrt, bias=eps[0:1, :])
            rstd = sm512_p.tile([1, SP], F32, tag="rcp", name="rstdf")
            nc.vector.reciprocal_approx_fast(out=rstd[:], in_=stdf[:])
            rstdB = bc_p.tile([128, C], F32, tag="bc", name="rstdB")
            nc.gpsimd.partition_broadcast(rstdB[:, 0:SP], rstd[:])
            cm = sm512_p.tile([1, SP], F32, tag="rcp", name="cm")
            nc.vector.tensor_tensor(cm[:], mean[:], rstd[:], op=OP.mult)
            c0 = sm512_p.tile([1, 1], F32, tag="c0", name="c0")
            nc.vector.tensor_reduce(c0[:], cm[:], axis=AX.X, op=OP.add)
            c0B = sm_p.tile([128, 1], F32, tag="c0b", name="c0B")
            nc.gpsimd.partition_broadcast(c0B[:], c0[:])

            ps_l = st_ps.tile([128, 512], F32, tag="st", name="ps_l")
            idx = 0
            for cc in range(6):
                for tb in range(TB):
                    hw = hw_p.tile([128, NCLS], BF16, tag="hw", name=f"hw{cc}_{tb}")
                    row0 = tb * C + cc * 128
                    nc.sync.dma_start(out=hw[:], in_=d_hwt[row0 : row0 + 128, :])
                    nc.tensor.matmul(
                        ps_l[0:NCLS, 0:SP],
                        hw[:],
                        hT[cc][:, tb * SP : (tb + 1) * SP],
                        start=(idx == 0),
                        stop=(idx == 47),
                    )
                    idx += 1
            gs = sm_p.tile([128, SP], F32, tag="gs", name="gs")
            nc.vector.tensor_tensor(gs[0:NCLS, :], ps_l[0:NCLS, 0:SP], rstdB[0:NCLS, 0:SP], op=OP.mult)
            red = sm_p.tile([128, 1], F32, tag="red", name="red")
            nc.vector.tensor_reduce(red[0:NCLS, :], gs[0:NCLS, :], axis=AX.X, op=OP.add)
            t1 = sm_p.tile([128, 1], F32, tag="t1", name="t1")
            nc.vector.tensor_scalar(
                t1[0:NCLS, :],
                cb[0:NCLS, CB_W1 : CB_W1 + 1],
                c0B[0:NCLS, :],
                None,
                op0=OP.mult,
            )
            t2 = sm_p.tile([128, 1], F32, tag="t2", name="t2")
            nc.vector.tensor_tensor(t2[0:NCLS, :], red[0:NCLS, :], t1[0:NCLS, :], op=OP.subtract)
            logits = sm_p.tile([128, 1], F32, tag="lg", name="logits")
            nc.vector.tensor_scalar(
                logits[0:NCLS, :],
                t2[0:NCLS, :],
                1.0 / SP,
                cb[0:NCLS, CB_HB : CB_HB + 1],
                op0=OP.mult,
                op1=OP.add,
            )
            nc.sync.dma_start(out=d_out[:], in_=logits[0:NCLS, :])

    nc.compile()
    return nc


_NC = None


def _get_nc():
    global _NC
    if _NC is None:
        _NC = _build()
    return _NC


def _prep_inputs(inputs):
    f = np.float32
    x = np.asarray(inputs["x"], f)
    B = x.shape[0]
    xpt = np.empty((B, PVEC, NTOK), bf16)
    for b in range(B):
        xp = x[b, 0].reshape(8, 8, 8, 8, 8, 8).transpose(0, 2, 4, 1, 3, 5).reshape(NTOK, PVEC)
        xpt[b] = np.ascontiguousarray(xp.T).astype(bf16)

    qw, kw, vw, pw = (np.asarray(inputs[k], f) for k in ("qw", "kw", "vw", "pw"))
    f1w, f2w = np.asarray(inputs["f1w"], f), np.asarray(inputs["f2w"], f)
    l1w, l1b = np.asarray(inputs["ln1_w"], f), np.asarray(inputs["ln1_b"], f)
    l2w, l2b = np.asarray(inputs["ln2_w"], f), np.asarray(inputs["ln2_b"], f)

    wq = np.ascontiguousarray((qw * l1w[:, None, :]).transpose(0, 2, 1)).astype(bf16)
    wk = np.ascontiguousarray((kw * l1w[:, None, :]).transpose(0, 2, 1)).astype(bf16)
    wv = np.ascontiguousarray((vw * l1w[:, None, :]).transpose(0, 2, 1)).astype(bf16)
    wp = np.ascontiguousarray(pw.transpose(0, 2, 1)).astype(bf16)
    w1 = np.ascontiguousarray((f1w * l2w[:, None, :]).transpose(0, 2, 1)).astype(bf16)
    w2 = np.ascontiguousarray(f2w.transpose(0, 2, 1)).astype(bf16)

    qb = np.asarray(inputs["qb"], f) + np.einsum("ioc,ic->io", qw, l1b)
    kb = np.asarray(inputs["kb"], f) + np.einsum("ioc,ic->io", kw, l1b)
    vb = np.asarray(inputs["vb"], f) + np.einsum("ioc,ic->io", vw, l1b)
    f1b = np.asarray(inputs["f1b"], f) + np.einsum("ijc,ic->ij", f1w, l2b)

    head_w = np.asarray(inputs["head_w"], f)
    fcn_w, fcn_b = np.asarray(inputs["fcn_w"], f), np.asarray(inputs["fcn_b"], f)
    head_b = np.asarray(inputs["head_b"], f) + head_w @ fcn_b
    hwt = np.ascontiguousarray(head_w.T * fcn_w[:, None])
    hwt_b = hwt.astype(bf16)

    cbp = np.zeros((128, CB_COLS), f)
    for i in range(DEPTH):
        c0 = i * CB_PER_LAYER
        cbp[:, c0 : c0 + 6] = qb[i].reshape(6, 128).T
        cbp[:, c0 + 6 : c0 + 12] = kb[i].reshape(6, 128).T
        cbp[:, c0 + 12 : c0 + 36] = f1b[i].reshape(24, 128).T
    cbp[:NCLS, CB_W1] = hwt_b.astype(f).sum(axis=0)
    cbp[:NCLS, CB_HB] = head_b

    rb = np.stack(
        [
            np.stack(
                [vb[i], np.asarray(inputs["pb"], f)[i], np.asarray(inputs["f2b"], f)[i]]
            )
            for i in range(DEPTH)
        ]
    ).astype(f)

    pos2 = (
        np.asarray(inputs["pos_embed"], f)[0] + np.asarray(inputs["patch_b"], f)[None, :]
    ).astype(f)
    pwt = np.ascontiguousarray(np.asarray(inputs["patch_w"], f).T).astype(bf16)

    shared = {
        "pos2": pos2,
        "pwt": pwt,
        "wq": wq,
        "wk": wk,
        "wv": wv,
        "wp": wp,
        "w1": w1,
        "w2": w2,
        "cb": cbp,
        "rb": rb,
        "hwt": hwt_b,
    }
    return xpt, shared


def kernel(**inputs):
    nc = _get_nc()
    xpt, shared = _prep_inputs(inputs)
    B = xpt.shape[0]
    in_maps = [dict(shared, xpt=xpt[b]) for b in range(B)]
    res = run_bass_kernel_spmd(nc, in_maps, list(range(B)))
    return np.stack([res.results[b]["out"] for b in range(B)]).astype(np.float32)


# revision 56
# speedup vs baseline: 1.0216x; 1.0216x over previous
"""HSIViT forward on 8 Trainium2 NeuronCores.

Sharding: pure data parallel — batch B=8, one batch item per core, no
collectives. Each core runs the full 12-layer ViT on its (512, 768)
token activations and emits its (100,) logits row.

Host-side prep (numpy, not counted in HW exec time):
  - patch cubes extracted + transposed per batch item (xpT [512, 512])
  - all weights transposed to [c_in, c_out] for the PE's lhsT layout
  - LN1/LN2 scale+bias folded into q/k/v and fc1 weights+biases
  - final feature-LN scale/bias folded into the classifier head
  - q/k/v/proj/MLP weights cast to bf16; patch embed + head ride fp32r

Scheduling notes (all aimed at keeping the PE queue dense so the
Tensor engine stays in its high p-state):
  - LN rstd via one ACT Rsqrt op; standardized rows evicted bf16 so the
    PE transposes run at 1 cyc/row; per-tile LN chains overlap the
    previous phase's matmuls (fc2/proj emit token-tile-ordered).
  - attention: S^T per head in 4 psum chunks -> ACT exp (bf16) -> AV
    with a ones-column in V producing the softmax denominator; AV lags
    scores by 2 heads and the V projection is interleaved after S0/S1
    so the PE has independent work while ACT exps drain.
  - softmax reciprocal via the custom-DVE fast-approx op; denominator
    broadcast on GpSimd; normalization fused into the oT eviction.
  - proj/fc2 biases pre-added into h during PE-busy windows so the
    post-psum residual is a single vector add.
"""

import os
import sys

import numpy as np

for _p in ("/opt/trn_rl_repo", "/root/.axon_site/_ro/trn_rl_repo"):
    if _p not in sys.path and os.path.isdir(_p):
        sys.path.insert(0, _p)

import ml_dtypes  # noqa: E402

import concourse.bass as bass  # noqa: E402,F401
import concourse.mybir as mybir  # noqa: E402
import concourse.tile as tile  # noqa: E402
from concourse import bacc  # noqa: E402
from concourse.bass_utils import run_bass_kernel_spmd  # noqa: E402
from concourse.masks import make_identity  # noqa: E402

F32 = mybir.dt.float32
F32R = mybir.dt.float32r
BF16 = mybir.dt.bfloat16
AF = mybir.ActivationFunctionType
OP = mybir.AluOpType
AX = mybir.AxisListType

DEPTH, C, NH, HD = 12, 768, 12, 64
NTOK, PVEC = 512, 512  # tokens, patch vector (8*8*8)
FF = 3072
NCLS = 100
TB, SP = 8, 64  # band groups, spatial positions
FD = TB * C  # 6144 final feature dim
SCALE = HD**-0.5
EPS = 1e-5

CB_PER_LAYER = 36  # qb 6 + kb 6 + f1b 24 columns
CB_W1 = DEPTH * CB_PER_LAYER  # head-weight column sums
CB_HB = CB_W1 + 1  # folded head bias
CB_COLS = CB_HB + 1

bf16 = ml_dtypes.bfloat16


def _build():
    nc = bacc.Bacc(None, target_bir_lowering=False, debug=False)

    d_xpt = nc.dram_tensor("xpt", [PVEC, NTOK], BF16, kind="ExternalInput")
    d_pos2 = nc.dram_tensor("pos2", [NTOK, C], F32, kind="ExternalInput")
    d_pwt = nc.dram_tensor("pwt", [PVEC, C], BF16, kind="ExternalInput")
    d_wq = nc.dram_tensor("wq", [DEPTH, C, C], BF16, kind="ExternalInput")
    d_wk = nc.dram_tensor("wk", [DEPTH, C, C], BF16, kind="ExternalInput")
    d_wv = nc.dram_tensor("wv", [DEPTH, C, C], BF16, kind="ExternalInput")
    d_wp = nc.dram_tensor("wp", [DEPTH, C, C], BF16, kind="ExternalInput")
    d_w1 = nc.dram_tensor("w1", [DEPTH, C, FF], BF16, kind="ExternalInput")
    d_w2 = nc.dram_tensor("w2", [DEPTH, FF, C], BF16, kind="ExternalInput")
    d_cb = nc.dram_tensor("cb", [128, CB_COLS], F32, kind="ExternalInput")
    d_rb = nc.dram_tensor("rb", [DEPTH, 3, C], F32, kind="ExternalInput")
    d_hwt = nc.dram_tensor("hwt", [FD, NCLS], BF16, kind="ExternalInput")
    d_out = nc.dram_tensor("out", [NCLS], F32, kind="ExternalOutput")

    from contextlib import ExitStack

    with tile.TileContext(nc) as tc:
        with ExitStack() as ctx:
            ep = ctx.enter_context
            const = ep(tc.tile_pool(name="const", bufs=1))
            hpool = ep(tc.tile_pool(name="hpool", bufs=4))
            arow_p = ep(tc.tile_pool(name="arow", bufs=4))
            aT_p = ep(tc.tile_pool(name="atp", bufs=6))
            a2T_p = ep(tc.tile_pool(name="a2tp", bufs=6))
            qT_p = ep(tc.tile_pool(name="qtp", bufs=6))
            kT_p = ep(tc.tile_pool(name="ktp", bufs=6))
            vx_p = ep(tc.tile_pool(name="vxp", bufs=4))
            ex_p = ep(tc.tile_pool(name="exp", bufs=14))
            oT_p = ep(tc.tile_pool(name="otp", bufs=6))
            gT_p = ep(tc.tile_pool(name="gtp", bufs=24))
            wqkv_p = ep(tc.tile_pool(name="wqkv", bufs=12))
            patch_p = ep(tc.tile_pool(name="patchp", bufs=4))
            hw_p = ep(tc.tile_pool(name="hwp", bufs=4))
            w1_p = ep(tc.tile_pool(name="w1p", bufs=6))
            w2_p = ep(tc.tile_pool(name="w2p", bufs=24))
            bc_p = ep(tc.tile_pool(name="bcp", bufs=2))
            rcp_p = ep(tc.tile_pool(name="rcpp", bufs=2))
            sm_p = ep(tc.tile_pool(name="smp", bufs=8))
            sm512_p = ep(tc.tile_pool(name="sm512", bufs=2))
            ftmp_p = ep(tc.tile_pool(name="ftmp", bufs=6))
            sq_p = ep(tc.tile_pool(name="sqp", bufs=2))
            mm_ps = ep(tc.tile_pool(name="mmps", bufs=3, space="PSUM"))
            st_ps = ep(tc.tile_pool(name="stps", bufs=3, space="PSUM"))
            tp_ps = ep(tc.tile_pool(name="tpps", bufs=2, space="PSUM"))

            ident = const.tile([128, 128], F32, tag="ident", name="ident")
            make_identity(nc, ident)
            identB = const.tile([128, 128], BF16, tag="identB", name="identB")
            nc.scalar.copy(identB[:], ident[:])
            ones0 = const.tile([128, 1], F32, tag="ones0", name="ones0")
            nc.vector.memset(ones0[:], 1.0)
            onesB = const.tile([128, 1], BF16, tag="onesB", name="onesB")
            nc.scalar.copy(onesB[:], ones0[:])
            eps = const.tile([128, 1], F32, tag="eps", name="eps")
            nc.vector.memset(eps[:], EPS)
            cb = const.tile([128, CB_COLS], F32, tag="cb", name="cb")
            nc.sync.dma_start(out=cb[:], in_=d_cb[:])

            h = []
            for t in range(4):
                ht = hpool.tile([128, C], F32, tag="h", name=f"h{t}")
                h.append(ht)

            def emit_stats0(t, tag):
                """First bn_stats half — emitted as soon as h[t][:, 0:384]
                is final (after the n=0 residual add) so it overlaps the
                n=1 psum matmuls."""
                st6 = sm_p.tile([128, 12], F32, tag="st6", name=f"st6_{tag}{t}")
                nc.vector.bn_stats(st6[:, 0:6], h[t][:, 0:384])
                return st6

            def emit_ln_rest(t, tag, st6):
                """Second stats half + (x - mean) * rsqrt(var + eps), bf16."""
                nc.vector.bn_stats(st6[:, 6:12], h[t][:, 384:768])
                mv = sm_p.tile([128, 2], F32, tag="mv", name=f"mv{tag}{t}")
                nc.vector.bn_aggr(mv[:], st6.rearrange("p (g s) -> p g s", g=2))
                std = sm_p.tile([128, 1], F32, tag="std", name=f"std{tag}{t}")
                nc.scalar.activation(std[:], mv[:, 1:2], AF.Sqrt, bias=eps[:])
                rstd = sm_p.tile([128, 1], F32, tag="rstd", name=f"rstd{tag}{t}")
                nc.vector.reciprocal_approx_fast(out=rstd[:], in_=std[:])
                at = arow_p.tile([128, C], BF16, tag="ar", name=f"ar{tag}{t}")
                nc.vector.tensor_scalar(
                    at[:], h[t], mv[:, 0:1], rstd[:], op0=OP.subtract, op1=OP.mult
                )
                return at

            # ---- patch embed: h = pos(+patch_b) + xp @ patch_w.T ----
            # pos is DMA'd straight into h, psum added on top; t-major with
            # layer 0's LN1 emitted inline so its chains overlap the PE.
            xpt = []
            pwt = []
            for kc in range(4):
                xt = aT_p.tile([128, NTOK], BF16, tag="at", name=f"xpt{kc}")
                nc.sync.dma_start(out=xt[:], in_=d_xpt[kc * 128 : (kc + 1) * 128, :])
                xpt.append(xt)
                wt = patch_p.tile([128, C], BF16, tag="pw", name=f"pwt{kc}")
                nc.sync.dma_start(out=wt[:], in_=d_pwt[kc * 128 : (kc + 1) * 128, :])
                pwt.append(wt)
            a_rows = []
            for t in range(4):
                nc.sync.dma_start(out=h[t][:], in_=d_pos2[t * 128 : (t + 1) * 128, :])
                st6 = None
                for n in range(2):
                    ns = slice(n * 384, (n + 1) * 384)
                    ps = mm_ps.tile([128, 512], F32, tag="mm", name=f"pep{t}{n}")
                    for kc in range(4):
                        nc.tensor.matmul(
                            ps[:, :384],
                            xpt[kc][:, t * 128 : (t + 1) * 128],
                            pwt[kc][:, ns],
                            start=(kc == 0),
                            stop=(kc == 3),
                        )
                    nc.vector.tensor_tensor(h[t][:, ns], h[t][:, ns], ps[:, :384], op=OP.add)
                    if n == 0:
                        st6 = emit_stats0(t, "a0_")
                a_rows.append(emit_ln_rest(t, "a0_", st6))

            def transpose_pass(rows, t_list, col0, outs, tag2):
                """Transpose the given token tiles' blocks into cols
                [col0 : col0 + 128*len(t_list)] of the 6 col tiles.
                Evictions alternate ACT/DVE so the tail drains on two
                queues."""
                w = 128 * len(t_list)
                for cc in range(6):
                    ps = tp_ps.tile([128, 512], BF16, tag="tp", name=f"tp{tag2}{cc}")
                    for ti, t in enumerate(t_list):
                        nc.tensor.transpose(
                            ps[:, ti * 128 : (ti + 1) * 128],
                            rows[t][:, cc * 128 : (cc + 1) * 128],
                            identB[:],
                        )
                    dst = outs[cc][:, col0 : col0 + w]
                    if cc % 2 == 0:
                        nc.scalar.copy(dst, ps[:, 0:w])
                    else:
                        nc.vector.tensor_copy(dst, ps[:, 0:w])

            def transpose_cols(rows, dst_pool, dst_tag):
                outs = [
                    dst_pool.tile([128, NTOK], BF16, tag=dst_tag, name=f"{dst_tag}{cc}")
                    for cc in range(6)
                ]
                transpose_pass(rows, (0, 1, 2, 3), 0, outs, dst_tag)
                return outs

            def bcast_row(i, j, tag):
                """rb[i, j] (768,) -> [128, 768] partition-broadcast tile."""
                src = sm512_p.tile([1, C], F32, tag="rbs", name=f"rbs{i}_{j}")
                nc.sync.dma_start(out=src[:], in_=d_rb[i, j])
                bt = bc_p.tile([128, C], F32, tag="bc", name=f"{tag}{i}")
                nc.gpsimd.partition_broadcast(bt[:], src[:])
                return bt

            for i in range(DEPTH):
                cb0 = i * CB_PER_LAYER
                # ---- transpose LN1 rows (tokens 0..383 first) ----
                aT = [
                    aT_p.tile([128, NTOK], BF16, tag="at", name=f"at{cc}")
                    for cc in range(6)
                ]
                transpose_pass(a_rows, (0, 1, 2), 0, aT, "at1_")

                # ---- q/k projections -> col layout [c_out, t], bf16;
                # two token passes: cols 0:384 run while tile 3's LN
                # finishes, then the t3 transpose pass, then cols 384:512 ----
                qk_w = []
                for (dw, tg) in ((d_wq, "qw"), (d_wk, "kw")):
                    wts = []
                    for kc in range(6):
                        wt = wqkv_p.tile([128, C], BF16, tag="wqkv", name=f"{tg}{kc}")
                        nc.sync.dma_start(out=wt[:], in_=dw[i, kc * 128 : (kc + 1) * 128, :])
                        wts.append(wt)
                    qk_w.append(wts)
                qT = [qT_p.tile([128, NTOK], BF16, tag="qt", name=f"qt{mc}") for mc in range(6)]
                kT = [kT_p.tile([128, NTOK], BF16, tag="kt", name=f"kt{mc}") for mc in range(6)]
                for cs, ce in ((0, 384), (384, 512)):
                    if cs == 384:
                        transpose_pass(a_rows, (3,), 384, aT, "at2_")
                    w = ce - cs
                    for mc in range(6):
                        for (wts, outs, base, tg) in (
                            (qk_w[0], qT, cb0, "qt"),
                            (qk_w[1], kT, cb0 + 6, "kt"),
                        ):
                            ps = mm_ps.tile([128, 512], F32, tag="mm", name=f"{tg}p{mc}")
                            for kc in range(6):
                                nc.tensor.matmul(
                                    ps[:, 0:w],
                                    wts[kc][:, mc * 128 : (mc + 1) * 128],
                                    aT[kc][:, cs:ce],
                                    start=(kc == 0),
                                    stop=(kc == 5),
                                )
                            nc.vector.tensor_scalar_add(
                                outs[mc][:, cs:ce], ps[:, 0:w],
                                cb[:, base + mc : base + mc + 1],
                            )

                # ---- attention, software-pipelined with the v projection ----
                vbB = bcast_row(i, 0, "vb")
                pbB = bcast_row(i, 1, "pb")
                vwts = []
                for kc in range(6):
                    wt = wqkv_p.tile([128, C], BF16, tag="wqkv", name=f"vw{kc}")
                    nc.sync.dma_start(out=wt[:], in_=d_wv[i, kc * 128 : (kc + 1) * 128, :])
                    vwts.append(wt)

                ex_all = [None] * NH
                po_all = [None] * NH
                oT = []
                for cc in range(6):
                    ot = oT_p.tile([128, NTOK], BF16, tag="ot", name=f"ot{cc}")
                    oT.append(ot)

                def emit_scores(hh):
                    pb_ = (hh % 2) * 64
                    qh = qT[hh // 2][pb_ : pb_ + 64, :]
                    kh = kT[hh // 2][pb_ : pb_ + 64, :]
                    exs = []
                    for j in range(4):
                        ps = st_ps.tile([128, 512], F32, tag="st", name=f"st{hh}_{j}")
                        nc.tensor.matmul(
                            ps[:],
                            kh[:, j * 128 : (j + 1) * 128],
                            qh,
                            start=True,
                            stop=True,
                        )
                        ex = ex_p.tile([128, NTOK], BF16, tag="ex", name=f"ex{hh}_{j}")
                        nc.scalar.activation(ex[:], ps[:], AF.Exp, scale=SCALE)
                        exs.append(ex)
                    ex_all[hh] = exs

                def emit_av(hh):
                    po = mm_ps.tile([128, 512], F32, tag="mm", name=f"po{hh}")
                    for j in range(4):
                        nc.tensor.matmul(
                            po[0:65, :],
                            v_ext[j].rearrange("p (h d) -> p h d", h=NH)[:, hh, :],
                            ex_all[hh][j][:],
                            start=(j == 0),
                            stop=(j == 3),
                        )
                    po_all[hh] = po
                    pb_ = (hh % 2) * 64
                    # custom-DVE ops misread PSUM; bounce the denom row to SBUF
                    den = sm512_p.tile([1, NTOK], F32, tag="rcp", name=f"den{hh}")
                    nc.vector.tensor_copy(den[:], po[64:65, :])
                    rcp = sm512_p.tile([1, NTOK], F32, tag="rcp", name=f"rcp{hh}")
                    nc.vector.reciprocal_approx_fast(out=rcp[:], in_=den[:])
                    rcpB = rcp_p.tile([64, NTOK], F32, tag="rb", name=f"rcpB{hh}")
                    nc.gpsimd.partition_broadcast(rcpB[:], rcp[:])
                    nc.vector.tensor_tensor(
                        oT[hh // 2][pb_ : pb_ + 64, :], po[0:64, :], rcpB[:], op=OP.mult
                    )

                def emit_vgroup(t):
                    vx = vx_p.tile([128, NH * (HD + 1)], BF16, tag="vx", name=f"vx{t}")
                    vxh = vx.rearrange("p (h d) -> p h d", h=NH)
                    for n in range(2):
                        ps = mm_ps.tile([128, 512], F32, tag="mm", name=f"vp{t}{n}")
                        for kc in range(6):
                            nc.tensor.matmul(
                                ps[:, :384],
                                aT[kc][:, t * 128 : (t + 1) * 128],
                                vwts[kc][:, n * 384 : (n + 1) * 384],
                                start=(kc == 0),
                                stop=(kc == 5),
                            )
                        nc.vector.tensor_tensor(
                            vxh[:, n * 6 : (n + 1) * 6, 0:HD],
                            ps[:, :384].rearrange("p (g d) -> p g d", g=6),
                            vbB[:, n * 384 : (n + 1) * 384].rearrange("p (g d) -> p g d", g=6),
                            op=OP.add,
                        )
                    nc.vector.memset(vxh[:, :, HD : HD + 1], 1.0)
                    v_ext.append(vx)

                # pipeline: S0 S1 [v x4] S2 AV0 S3 AV1 ... S11 AV9 AV10 AV11
                # — the v projection fills the PE while the first exps drain
                # on ACT; AV lags scores by 2 heads to hide exp latency.
                v_ext = []
                emit_scores(0)
                emit_scores(1)
                for t in range(4):
                    emit_vgroup(t)
                for t in range(4):
                    nc.vector.tensor_tensor(h[t][:], h[t][:], pbB[:], op=OP.add)
                for hh in range(2, NH):
                    emit_scores(hh)
                    emit_av(hh - 2)
                emit_av(NH - 2)
                emit_av(NH - 1)

                # ---- output projection + residual (pb pre-added), t-major
                # with LN2 emitted inline per tile ----
                pwts = []
                for kc in range(6):
                    wt = wqkv_p.tile([128, C], BF16, tag="wqkv", name=f"pw{kc}")
                    nc.sync.dma_start(out=wt[:], in_=d_wp[i, kc * 128 : (kc + 1) * 128, :])
                    pwts.append(wt)
                a2_rows = []
                for t in range(4):
                    st6 = None
                    for n in range(2):
                        ns = slice(n * 384, (n + 1) * 384)
                        ps = mm_ps.tile([128, 512], F32, tag="mm", name=f"prj{t}{n}")
                        for kc in range(6):
                            nc.tensor.matmul(
                                ps[:, :384],
                                oT[kc][:, t * 128 : (t + 1) * 128],
                                pwts[kc][:, ns],
                                start=(kc == 0),
                                stop=(kc == 5),
                            )
                        nc.vector.tensor_tensor(h[t][:, ns], h[t][:, ns], ps[:, :384], op=OP.add)
                        if n == 0:
                            st6 = emit_stats0(t, f"b{i}_")
                    a2_rows.append(emit_ln_rest(t, f"b{i}_", st6))
                a2T = [
                    a2T_p.tile([128, NTOK], BF16, tag="a2t", name=f"a2t{cc}")
                    for cc in range(6)
                ]
                transpose_pass(a2_rows, (0, 1, 2), 0, a2T, "a2t1_")

                # ---- fc1 + gelu -> gT col layout [j, t] bf16; the first
                # half's 0:384 pass hides tile 3's LN + transpose ----
                f2bB = bcast_row(i, 2, "fb")
                gT = [gT_p.tile([128, NTOK], BF16, tag="gt", name=f"gt{m}") for m in range(24)]
                for half in range(2):
                    wts = []
                    for kc in range(6):
                        wt = w1_p.tile([128, FF // 2], BF16, tag="w1", name=f"w1_{half}_{kc}")
                        nc.sync.dma_start(
                            out=wt[:],
                            in_=d_w1[
                                i,
                                kc * 128 : (kc + 1) * 128,
                                half * (FF // 2) : (half + 1) * (FF // 2),
                            ],
                        )
                        wts.append(wt)
                    for cs, ce in ((0, 384), (384, 512)):
                        if cs == 384 and half == 0:
                            transpose_pass(a2_rows, (3,), 384, a2T, "a2t2_")
                        w = ce - cs
                        for mh in range(12):
                            m = half * 12 + mh
                            ps = mm_ps.tile([128, 512], F32, tag="mm", name=f"f1p{m}")
                            for kc in range(6):
                                nc.tensor.matmul(
                                    ps[:, 0:w],
                                    wts[kc][:, mh * 128 : (mh + 1) * 128],
                                    a2T[kc][:, cs:ce],
                                    start=(kc == 0),
                                    stop=(kc == 5),
                                )
                            nc.scalar.activation(
                                gT[m][:, cs:ce], ps[:, 0:w], AF.Gelu,
                                bias=cb[:, cb0 + 12 + m : cb0 + 13 + m],
                            )
                        if half == 0 and cs == 0:
                            # f2b pre-add rides the fc1 window
                            for t in range(4):
                                nc.vector.tensor_tensor(h[t][:], h[t][:], f2bB[:], op=OP.add)

                # ---- fc2 + residual (f2b pre-added); t-major with the NEXT
                # layer's LN1 emitted inline per tile (the LN chains overlap
                # the remaining tiles' fc2 matmuls) ----
                w2ts = []
                for jc in range(24):
                    wt = w2_p.tile([128, C], BF16, tag="w2", name=f"w2_{jc}")
                    nc.sync.dma_start(out=wt[:], in_=d_w2[i, jc * 128 : (jc + 1) * 128, :])
                    w2ts.append(wt)
                a_rows = []
                for t in range(4):
                    st6 = None
                    for n in range(2):
                        ns = slice(n * 384, (n + 1) * 384)
                        ps = mm_ps.tile([128, 512], F32, tag="mm", name=f"f2p{t}{n}")
                        for jc in range(24):
                            nc.tensor.matmul(
                                ps[:, :384],
                                gT[jc][:, t * 128 : (t + 1) * 128],
                                w2ts[jc][:, ns],
                                start=(jc == 0),
                                stop=(jc == 23),
                            )
                        nc.vector.tensor_tensor(h[t][:, ns], h[t][:, ns], ps[:, :384], op=OP.add)
                        if n == 0 and i < DEPTH - 1:
                            st6 = emit_stats0(t, f"a{i + 1}_")
                    if i < DEPTH - 1:
                        a_rows.append(emit_ln_rest(t, f"a{i + 1}_", st6))

            # ---- final: transpose h, feature-LN stats, head ----
            # Per spatial s, feat[s, :] is LN'd over f in [0, 6144) with the
            # LN scale/bias already folded into hwt/head_b. Standardization is
            # folded PAST the head matmul:
            #   logits[n] = (1/64) sum_s rstd[s]*G[n,s]
            #             - (1/64)(sum_s rstd[s]*mean[s]) * W1[n] + head_b'[n]
            # with G = hwt^T @ featT and W1[n] = sum_f hwt[f, n].
            hb = []
            for t in range(4):
                hbt = arow_p.tile([128, C], BF16, tag="ar", name=f"hb{t}")
                nc.scalar.copy(hbt[:], h[t][:])
                hb.append(hbt)
            hT = transpose_cols(hb, ftmp_p, "ht")
            ps_s = st_ps.tile([128, 512], F32, tag="st", name="ps_s")
            ps_q = st_ps.tile([128, 512], F32, tag="st", name="ps_q")
            for cc in range(6):
                s = sq_p.tile([128, NTOK], BF16, tag="sq", name=f"sq{cc}")
                nc.scalar.activation(s[:], hT[cc][:], AF.Square)
                for tb in range(TB):
                    idx = cc * TB + tb
                    nc.tensor.matmul(
                        ps_s[0:1, 0:SP],
                        onesB[:],
                        hT[cc][:, tb * SP : (tb + 1) * SP],
                        start=(idx == 0),
                        stop=(idx == 47),
                    )
                for tb in range(TB):
                    idx = cc * TB + tb
                    nc.tensor.matmul(
                        ps_q[0:1, 0:SP],
                        onesB[:],
                        s[:, tb * SP : (tb + 1) * SP],
                        start=(idx == 0),
                        stop=(idx == 47),
                    )
            mean = sm512_p.tile([1, SP], F32, tag="rbs", name="mean")
            nc.vector.tensor_scalar_mul(mean[:], ps_s[0:1, 0:SP], 1.0 / FD)
            msq = sm512_p.tile([1, SP], F32, tag="rbs", name="msq")
            nc.vector.tensor_scalar_mul(msq[:], ps_q[0:1, 0:SP], 1.0 / FD)
            mm2 = sm512_p.tile([1, SP], F32, tag="rcp", name="mm2")
            nc.vector.tensor_tensor(mm2[:], mean[:], mean[:], op=OP.mult)
            var = sm512_p.tile([1, SP], F32, tag="rcp", name="var")
            nc.vector.tensor_tensor(var[:], msq[:], mm2[:], op=OP.subtract)
            stdf = sm512_p.tile([1, SP], F32, tag="rcp", name="stdf")
            nc.scalar.activation(stdf[:], var[:], AF.Sqrt, bias=eps[0:1, :])
            rstd = sm512_p.tile([1, SP], F32, tag="rcp", name="rstdf")
            nc.vector.reciprocal_approx_fast(out=rstd[:], in_=stdf[:])
            rstdB = bc_p.tile([128, C], F32, tag="bc", name="rstdB")
            nc.gpsimd.partition_broadcast(rstdB[:, 0:SP], rstd[:])
            cm = sm512_p.tile([1, SP], F32, tag="rcp", name="cm")
            nc.vector.tensor_tensor(cm[:], mean[:], rstd[:], op=OP.mult)
            c0 = sm512_p.tile([1, 1], F32, tag="c0", name="c0")
            nc.vector.tensor_reduce(c0[:], cm[:], axis=AX.X, op=OP.add)
            c0B = sm_p.tile([128, 1], F32, tag="c0b", name="c0B")
            nc.gpsimd.partition_broadcast(c0B[:], c0[:])

            ps_l = st_ps.tile([128, 512], F32, tag="st", name="ps_l")
            idx = 0
            for cc in range(6):
                for tb in range(TB):
                    hw = hw_p.tile([128, NCLS], BF16, tag="hw", name=f"hw{cc}_{tb}")
                    row0 = tb * C + cc * 128
                    nc.sync.dma_start(out=hw[:], in_=d_hwt[row0 : row0 + 128, :])
                    nc.tensor.matmul(
                        ps_l[0:NCLS, 0:SP],
                        hw[:],
                        hT[cc][:, tb * SP : (tb + 1) * SP],
                        start=(idx == 0),
                        stop=(idx == 47),
                    )
                    idx += 1
            gs = sm_p.tile([128, SP], F32, tag="gs", name="gs")
            nc.vector.tensor_tensor(gs[0:NCLS, :], ps_l[0:NCLS, 0:SP], rstdB[0:NCLS, 0:SP], op=OP.mult)
            red = sm_p.tile([128, 1], F32, tag="red", name="red")
            nc.vector.tensor_reduce(red[0:NCLS, :], gs[0:NCLS, :], axis=AX.X, op=OP.add)
            t1 = sm_p.tile([128, 1], F32, tag="t1", name="t1")
            nc.vector.tensor_scalar(
                t1[0:NCLS, :],
                cb[0:NCLS, CB_W1 : CB_W1 + 1],
                c0B[0:NCLS, :],
                None,
                op0=OP.mult,
            )
            t2 = sm_p.tile([128, 1], F32, tag="t2", name="t2")
            nc.vector.tensor_tensor(t2[0:NCLS, :], red[0:NCLS, :], t1[0:NCLS, :], op=OP.subtract)
            logits = sm_p.tile([128, 1], F32, tag="lg", name="logits")
            nc.vector.tensor_scalar(
                logits[0:NCLS, :],
                t2[0:NCLS, :],
                1.0 / SP,
                cb[0:NCLS, CB_HB : CB_HB + 1],
                op0=OP.mult,
                op1=OP.add,
            )
            nc.sync.dma_start(out=d_out[:], in_=logits[0:NCLS, :])

    nc.compile()
    return nc


_NC = None


def _get_nc():
    global _NC
    if _NC is None:
        _NC = _build()
    return _NC


def _prep_inputs(inputs):
    f = np.float32
    x = np.asarray(inputs["x"], f)
    B = x.shape[0]
    xpt = np.empty((B, PVEC, NTOK), bf16)
    for b in range(B):
        xp = x[b, 0].reshape(8, 8, 8, 8, 8, 8).transpose(0, 2, 4, 1, 3, 5).reshape(NTOK, PVEC)
        xpt[b] = np.ascontiguousarray(xp.T).astype(bf16)

    qw, kw, vw, pw = (np.asarray(inputs[k], f) for k in ("qw", "kw", "vw", "pw"))
    f1w, f2w = np.asarray(inputs["f1w"], f), np.asarray(inputs["f2w"], f)
    l1w, l1b = np.asarray(inputs["ln1_w"], f), np.asarray(inputs["ln1_b"], f)
    l2w, l2b = np.asarray(inputs["ln2_w"], f), np.asarray(inputs["ln2_b"], f)

    wq = np.ascontiguousarray((qw * l1w[:, None, :]).transpose(0, 2, 1)).astype(bf16)
    wk = np.ascontiguousarray((kw * l1w[:, None, :]).transpose(0, 2, 1)).astype(bf16)
    wv = np.ascontiguousarray((vw * l1w[:, None, :]).transpose(0, 2, 1)).astype(bf16)
    wp = np.ascontiguousarray(pw.transpose(0, 2, 1)).astype(bf16)
    w1 = np.ascontiguousarray((f1w * l2w[:, None, :]).transpose(0, 2, 1)).astype(bf16)
    w2 = np.ascontiguousarray(f2w.transpose(0, 2, 1)).astype(bf16)

    qb = np.asarray(inputs["qb"], f) + np.einsum("ioc,ic->io", qw, l1b)
    kb = np.asarray(inputs["kb"], f) + np.einsum("ioc,ic->io", kw, l1b)
    vb = np.asarray(inputs["vb"], f) + np.einsum("ioc,ic->io", vw, l1b)
    f1b = np.asarray(inputs["f1b"], f) + np.einsum("ijc,ic->ij", f1w, l2b)

    head_w = np.asarray(inputs["head_w"], f)
    fcn_w, fcn_b = np.asarray(inputs["fcn_w"], f), np.asarray(inputs["fcn_b"], f)
    head_b = np.asarray(inputs["head_b"], f) + head_w @ fcn_b
    hwt = np.ascontiguousarray(head_w.T * fcn_w[:, None])
    hwt_b = hwt.astype(bf16)

    cbp = np.zeros((128, CB_COLS), f)
    for i in range(DEPTH):
        c0 = i * CB_PER_LAYER
        cbp[:, c0 : c0 + 6] = qb[i].reshape(6, 128).T
        cbp[:, c0 + 6 : c0 + 12] = kb[i].reshape(6, 128).T
        cbp[:, c0 + 12 : c0 + 36] = f1b[i].reshape(24, 128).T
    cbp[:NCLS, CB_W1] = hwt_b.astype(f).sum(axis=0)
    cbp[:NCLS, CB_HB] = head_b

    rb = np.stack(
        [
            np.stack(
                [vb[i], np.asarray(inputs["pb"], f)[i], np.asarray(inputs["f2b"], f)[i]]
            )
            for i in range(DEPTH)
        ]
    ).astype(f)

    pos2 = (
        np.asarray(inputs["pos_embed"], f)[0] + np.asarray(inputs["patch_b"], f)[None, :]
    ).astype(f)
    pwt = np.ascontiguousarray(np.asarray(inputs["patch_w"], f).T).astype(bf16)

    shared = {
        "pos2": pos2,
        "pwt": pwt,
        "wq": wq,
        "wk": wk,
        "wv": wv,
        "wp": wp,
        "w1": w1,
        "w2": w2,
        "cb": cbp,
        "rb": rb,
        "hwt": hwt_b,
    }
    return xpt, shared


def kernel(**inputs):
    nc = _get_nc()
    xpt, shared = _prep_inputs(inputs)
    B = xpt.shape[0]
    in_maps = [dict(shared, xpt=xpt[b]) for b in range(B)]
    res = run_bass_kernel_spmd(nc, in_maps, list(range(B)))
    return np.stack([res.results[b]["out"] for b in range(B)]).astype(np.float32)
